# revision 16
# baseline (speedup 1.0000x reference)
"""Trainium2 Bass kernel for nn_DynamicComposeBlock.

Math (per (b,t)):
    out[o,h,w] = (sum_c W3d[o,c]*th[c,h]*tw[c,w] + b3d[o]) * (1-heat)*mask
                 + (sum_c W1d[o,c]*obj[c] + b1d[o]) * heat*mask

Key identity: with A = (1-heat)*mask and hm = heat*mask (functions of (h,w)
only), the blend commutes through the channel contraction:
    (W @ M) * A = W @ (M * A)        [M = th (x) tw outer product]
so the kernel computes M' = (th (x) tw) * A on the vector engine and a single
accumulated matmul  psum[o,hw] = W3dT.T @ M' + b3d (x) A + u (x) hm  on the
tensor engine, where u = W1d @ fea_obj + b1d (host-computed, tiny). The
rank-1 terms ride in a zero-padded K=128 matmul: TRN2's PE clock gate (HAM)
only sustains the fast clock for full-K matmuls, so every matmul here is
K=128. The psum->sbuf evacuation is then a plain copy (ACT engine).

Sharding: the 32 (b,t) pairs are split 4 per core across 8 cores; the small
weights are replicated. Each core writes its disjoint [4, 256, 64*64] slice.
"""
import os
import sys

for _p in ("/opt/trn_rl_repo",):
    if _p not in sys.path:
        sys.path.insert(0, _p)

import numpy as np

import concourse.bass as bass
import concourse.tile as tile
from concourse import bacc, mybir
from concourse.bass_utils import run_bass_kernel_spmd

N_CORES = 8
B, C, O, T, H, W = 2, 256, 256, 16, 64, 64
HW = H * W                      # 4096
JB = (B * T) // N_CORES         # 4 (b,t) pairs per core
KC = C // 128                   # 2 contraction chunks
OC = O // 128                   # 2 output-channel chunks

F32 = mybir.dt.float32
F16 = mybir.dt.float16

TRACE = {"on": False}  # test.py flips this to get HW exec time
USE_F16 = True


def build_nc():
    nc = bacc.Bacc("TRN2", target_bir_lowering=False, debug=False)

    def din(name, shape, dt=F16):
        return nc.dram_tensor(name, shape, dt, kind="ExternalInput").ap()

    th2_d = din("th2", [JB, C, H, 2])      # th duplicated in pairs (DVE 2x mode)
    tw_d = din("twf", [JB, C, W])
    w3_d = din("w3m", [C, O])              # W3d.T
    rows_d = din("rows", [JB, 2, HW])      # [A; hm] per (b,t)
    lx_d = din("lxp", [JB, 128, O])        # [b3d; u_j; zeros...] per (b,t)
    z_d = din("z128", [128, HW])           # zeros
    op_d = din("opad", [128, 128])         # row0 = ones, rest zeros
    out_d = nc.dram_tensor("out", [JB, O, HW], F32, kind="ExternalOutput").ap()

    with tile.TileContext(nc) as tc:
        with (
            tc.tile_pool(name="const", bufs=1) as pconst,
            tc.tile_pool(name="pin", bufs=3) as pin,
            tc.tile_pool(name="prow", bufs=3) as prow,
            tc.tile_pool(name="pam", bufs=2) as pam,
            tc.tile_pool(name="pm", bufs=2) as pm,
            tc.tile_pool(name="pmp", bufs=2) as pmp,
            tc.tile_pool(name="posb", bufs=3) as posb,
            tc.tile_pool(name="psa", bufs=2, space="PSUM") as psa,
            tc.tile_pool(name="pso", bufs=2, space="PSUM") as pso,
        ):
            # ---- constants (loaded once) ----
            opad = pconst.tile([128, 128], F16)
            nc.gpsimd.dma_start(opad[:], op_d[:])
            # rx slots: rows 0-1 overwritten per (b,t); rows 2-127 stay zero
            rx0 = pconst.tile([128, HW], F16, tag="rx0")
            rx1 = pconst.tile([128, HW], F16, tag="rx1")
            rx2 = pconst.tile([128, HW], F16, tag="rx2")
            rx = [rx0, rx1, rx2]
            nc.gpsimd.dma_start(rx0[:], z_d[:])
            w3 = pconst.tile([128, KC, O], F16)

            lxps = {}
            areps = {}
            ths = {}
            tws = {}

            def prep(j):
                """rows/lxp loads + A_rep broadcast for iteration j."""
                rxj = rx[j % 3]
                nc.gpsimd.dma_start(rxj[0:2, :], rows_d[j])
                lxp = prow.tile([128, O], F16, tag="lxp")
                nc.gpsimd.dma_start(lxp[:], lx_d[j])
                lxps[j] = lxp
                th2 = pin.tile([128, KC, H, 2], F16, tag="th2")
                nc.gpsimd.dma_start(
                    th2[:], th2_d[j].rearrange("(k p) h two -> p k h two", p=128)
                )
                ths[j] = th2
                twt = pin.tile([128, KC, W], F16, tag="twt")
                nc.gpsimd.dma_start(
                    twt[:], tw_d[j].rearrange("(k p) w -> p k w", p=128)
                )
                tws[j] = twt
                arep = pam.tile([128, HW], F16, tag="arep")
                for q in range(HW // 1024):
                    psq = psa.tile([128, 1024], F32, tag="psq_a")
                    for hh in range(2):
                        sl = slice(q * 1024 + hh * 512, q * 1024 + hh * 512 + 512)
                        nc.tensor.matmul(
                            psq[:, hh * 512 : hh * 512 + 512],
                            opad[:], rxj[:, sl],
                            start=True, stop=True,
                        )
                    if q < 2:
                        nc.vector.tensor_copy(
                            arep[:, q * 1024 : (q + 1) * 1024], psq[:]
                        )
                    else:
                        nc.scalar.copy(arep[:, q * 1024 : (q + 1) * 1024], psq[:])
                areps[j] = arep

            prep(0)
            nc.gpsimd.dma_start(w3[:], w3_d.rearrange("(k p) o -> p k o", p=128))
            nc.gpsimd.dma_start(rx1[:], z_d[:])
            nc.gpsimd.dma_start(rx2[:], z_d[:])
            for j in range(JB):
                if j + 1 < JB:
                    prep(j + 1)
                rxj = rx[j % 3]
                th2, twt, lxp, arep = ths[j], tws[j], lxps[j], areps[j]

                # ---- M' = (th (x) tw) * A per contraction chunk ----
                mp = pmp.tile([128, KC, HW], F16)
                for k in range(KC):
                    mk = pm.tile([128, HW], F16)
                    i0 = th2[:, k].unsqueeze(2).broadcast_to([128, H, W // 2, 2])
                    i1 = (
                        twt[:, k].unsqueeze(1).broadcast_to([128, H, W])
                        .rearrange("p h (a b) -> p h a b", b=2)
                    )
                    mo = mk[:].rearrange("p (h a b) -> p h a b", h=H, b=2)
                    nc.vector.tensor_mul(mo, i0, i1)
                    nc.vector.tensor_mul(mp[:, k, :], mk[:], arep[:])

                # ---- psum[o, hw] = W3dT.T @ M' + rank-1 terms, evac, store ----
                for oc in range(OC):
                    osb = posb.tile([128, HW], F32)
                    osl = slice(oc * 128, oc * 128 + 128)
                    for t2 in range(HW // 1024):
                        psq = pso.tile([128, 1024], F32)
                        for hh in range(2):
                            nsl = slice(t2 * 1024 + hh * 512, t2 * 1024 + hh * 512 + 512)
                            pslice = psq[:, hh * 512 : hh * 512 + 512]
                            nc.tensor.matmul(
                                pslice, w3[:, 0, osl], mp[:, 0, nsl],
                                start=True, stop=False,
                            )
                            nc.tensor.matmul(
                                pslice, w3[:, 1, osl], mp[:, 1, nsl],
                                start=False, stop=False,
                            )
                            nc.tensor.matmul(
                                pslice, lxp[:, osl], rxj[:, nsl],
                                start=False, stop=True,
                            )
                        nc.scalar.copy(
                            osb[:, t2 * 1024 : (t2 + 1) * 1024], psq[:]
                        )
                    nc.sync.dma_start(
                        out_d[j, osl, 0:2048], osb[:, 0:2048]
                    )
                    nc.scalar.dma_start(
                        out_d[j, osl, 2048:4096], osb[:, 2048:4096]
                    )

    nc.compile()
    return nc


_NC_CACHE = {}


def _get_nc():
    if "nc" not in _NC_CACHE:
        _NC_CACHE["nc"] = build_nc()
    return _NC_CACHE["nc"]


def kernel(fea_th, fea_tw, fea_obj, heatmap, mask, W3d, b3d, W1d, b1d):
    fea_th = np.asarray(fea_th, np.float32)
    fea_tw = np.asarray(fea_tw, np.float32)
    fea_obj = np.asarray(fea_obj, np.float32)
    heatmap = np.asarray(heatmap, np.float32)
    mask = np.asarray(mask, np.float32)
    W3d = np.asarray(W3d, np.float32)
    b3d = np.asarray(b3d, np.float32).reshape(O)
    b1d = np.asarray(b1d, np.float32).reshape(O)
    W1d = np.asarray(W1d, np.float32)
    w3m = np.ascontiguousarray(W3d.T).astype(np.float16)

    heat_f = heatmap[:, 0].reshape(B * T, HW)
    mask_f = mask[:, 0].reshape(B * T, HW)
    arow_f = ((1.0 - heat_f) * mask_f).astype(np.float16)
    hmrow_f = (heat_f * mask_f).astype(np.float16)
    # u[bt, o] = W1d @ fea_obj[bt] + b1d  (tiny; host-side)
    u_all = (
        np.einsum("oc,bct->bto", W1d, fea_obj, optimize=True)
        + b1d[None, None, :]
    ).reshape(B * T, O)

    nc = _get_nc()
    zeros128 = np.zeros((128, HW), np.float16)
    opad = np.concatenate(
        [np.ones((1, 128), np.float16), np.zeros((127, 128), np.float16)]
    )
    in_maps = []
    for core in range(N_CORES):
        bts = [divmod(core * JB + j, T) for j in range(JB)]
        bti = [b * T + t for b, t in bts]
        th = np.stack([fea_th[b, :, t, :] for b, t in bts])       # [JB, C, H]
        tw = np.stack([fea_tw[b, :, t, :] for b, t in bts])       # [JB, C, W]
        lxp = np.zeros((JB, 128, O), np.float16)
        for j, i in enumerate(bti):
            lxp[j, 0] = b3d.astype(np.float16)
            lxp[j, 1] = u_all[i].astype(np.float16)
        m = {
            "th2": np.ascontiguousarray(
                np.repeat(th.astype(np.float16)[..., None], 2, axis=-1)
            ),
            "twf": np.ascontiguousarray(tw.astype(np.float16)),
            "w3m": w3m,
            "rows": np.ascontiguousarray(
                np.stack([np.stack([arow_f[i], hmrow_f[i]]) for i in bti])
            ),
            "lxp": lxp,
            "z128": zeros128,
            "opad": opad,
        }
        in_maps.append(m)

    res = run_bass_kernel_spmd(
        nc, in_maps, core_ids=list(range(N_CORES)), trace=TRACE["on"]
    )
    if TRACE["on"]:
        TRACE["exec_time_ns"] = res.exec_time_ns
        TRACE["mean_exec_time_ns"] = res.mean_exec_time_ns
        TRACE["trace_path"] = (
            res.instructions_and_trace[1] if res.instructions_and_trace else None
        )

    out = np.empty((B, O, T, H, W), np.float32)
    for core in range(N_CORES):
        o = res.results[core]["out"]                               # [JB, O, HW]
        for j in range(JB):
            b, t = divmod(core * JB + j, T)
            out[b, :, t] = o[j].reshape(O, H, W)
    return out
